# revision 26
# baseline (speedup 1.0000x reference)
import numpy as np
import ml_dtypes

N = 50000
F = 64
E = 128
Q = 8
S = 2048
NC = 8
NPC = N // NC          # 6250 clauses per core
NPAD = 6400            # 25 * 256
NSC = 25               # mask DMA super-chunks (2 x 128 clauses each)
NCHUNK = NPAD // 128   # 50
SB = 4                 # psum banks of 512 steps each
ENTROPY_COEF = 0.1

_PROG = None


def _build_prog():
    import sys
    if "/opt/trn_rl_repo" not in sys.path:
        sys.path.insert(0, "/opt/trn_rl_repo")
    from concourse import bass, bacc, tile, mybir

    f32 = mybir.dt.float32
    bf16 = mybir.dt.bfloat16
    f8 = mybir.dt.float8e4
    AF = mybir.ActivationFunctionType
    ALU = mybir.AluOpType

    # Bacc (not plain Bass): finalize() runs compile() passes incl.
    # generate_event_semaphores, which splits multi-wait instructions into
    # InstEventSemaphore chains -- required, HW allows 1 wait/instruction.
    nc = bacc.Bacc("TRN2")
    # fw = [W1 | fvT] on 64 partitions; wb = [W2T | keysT | b1 | b2] on 128.
    # Packing each matmul's fresh operands into ONE DMA keeps every Matmult
    # at <=1 sync wait (HW limit; 2 waits crashes neuronxcc codegen).
    fw_d = nc.dram_tensor("fw", [F, E + NPAD], f32, kind="ExternalInput")
    wb_d = nc.dram_tensor("wb", [E, E + Q + 2], f32, kind="ExternalInput")
    maskT_d = nc.dram_tensor("maskT", [128, NSC, 2, S], f8, kind="ExternalInput")
    stats_d = nc.dram_tensor("stats", [17, S], f32, kind="ExternalOutput")
    xall_d = nc.dram_tensor("xall", [E, NCHUNK * Q], f32, kind="ExternalOutput")

    with tile.TileContext(nc) as tc:
        with (
            tc.tile_pool(name="const", bufs=1) as constp,
            tc.tile_pool(name="big", bufs=1) as bigp,
            tc.tile_pool(name="mask", bufs=1) as maskp,
            tc.tile_pool(name="ps", bufs=1, space=bass.MemorySpace.PSUM) as ps,
        ):
            def wt():
                return ps.tile([E, 512], f32, tag="w", bufs=3, name="w")

            wb_sb = constp.tile([E, E + Q + 2], f32)
            ones_sb = constp.tile([1, E], f32)
            k2t_sb = constp.tile([E, Q], f32)
            cq_sb = constp.tile([1, Q], f32)

            fw_sb = bigp.tile([F, E + NPAD], f32)
            ht_sb = bigp.tile([E, NPAD], f32)
            etmp_sb = bigp.tile([E, NCHUNK, Q], f32)
            scr_sb = constp.tile([1, 1], f32)
            stat_sb = bigp.tile([E, NCHUNK, 17], bf16)
            xall_sb = bigp.tile([E, NCHUNK * Q], f32)
            stats_sb = bigp.tile([17, SB * 512], f32)

            w1_sb = fw_sb[:, 0:E]
            fvt_sb = fw_sb[:, E:E + NPAD]
            w2t_sb = wb_sb[:, 0:E]
            keyst_sb = wb_sb[:, E:E + Q]
            b1_sb = wb_sb[:, E + Q:E + Q + 1]
            b2_sb = wb_sb[:, E + Q + 1:E + Q + 2]

            nc.scalar.dma_start(wb_sb[:], wb_d[:])
            nc.scalar.dma_start(fw_sb[:], fw_d[:])
            nc.vector.memset(ones_sb[:], 1.0)
            nc.vector.memset(stat_sb[:, :, 16], 1.0)
            # ACT-engine absorber: soaks up the wb DMA semaphore so later ACT
            # ops carry a single sync wait (HW limit per compute instruction)
            nc.scalar.activation(scr_sb[:], wb_sb[0:1, 0:1], AF.Relu)

            # <=8 total DMA instructions -> every DMA gets a unique DMAHW
            # lane, so no lane-reuse wait; each DMACopy carries at most its
            # single data wait (HW limit). Mask split across both physical
            # HWDGE rings (SP + ACT) for bandwidth.
            MB = [(0, 8, nc.sync), (8, 16, nc.sync), (16, NSC, nc.scalar)]
            mts = []
            for s0, s1, eng in MB:
                mt = maskp.tile([128, s1 - s0, 2, S], f8, tag=f"m{s0}")
                eng.dma_start(mt[:], maskT_d[:, s0:s1, :, :])
                mts.append(mt)

            def mslice(k, b):
                sc, half = k // 2, k % 2
                for (s0, s1, _), mt in zip(MB, mts):
                    if s0 <= sc < s1:
                        return mt[:, sc - s0, half, 512 * b:512 * (b + 1)]

            # K2T[e,q] = (W2 @ keys.T)[e,q]; c[q] = b2 . keys[q]
            pk2 = wt()
            nc.tensor.matmul(pk2[:, 0:Q], w2t_sb[:], keyst_sb[:], start=True, stop=True)
            nc.vector.tensor_copy(k2t_sb[:], pk2[:, 0:Q])
            pc = wt()
            nc.tensor.matmul(pc[0:1, 0:Q], b2_sb[:], keyst_sb[:], start=True, stop=True)
            nc.vector.tensor_copy(cq_sb[:], pc[0:1, 0:Q])

            # hT = relu(W1.T @ fvT + b1)   [E, NPAD]
            for j in range((NPAD + 511) // 512):
                c0 = 512 * j
                cw = min(512, NPAD - c0)
                ph = wt()
                nc.tensor.matmul(ph[:, :cw], w1_sb[:], fvt_sb[:, c0:c0 + cw],
                                 start=True, stop=True)
                nc.scalar.activation(ht_sb[:, c0:c0 + cw], ph[:, :cw], AF.Relu,
                                     bias=b1_sb[:], scale=1.0)

            # per 128-clause chunk: x[n,q] = hT_chunk.T @ K2T + c ; E=exp(x); G=E*x
            for k in range(NCHUNK):
                px = wt()
                nc.tensor.matmul(px[:, 0:Q], ht_sb[:, 128 * k:128 * (k + 1)], k2t_sb[:],
                                 start=True, stop=False)
                nc.tensor.matmul(px[:, 0:Q], ones_sb[:], cq_sb[:], start=False, stop=True)
                nc.scalar.activation(etmp_sb[:, k, :], px[:, 0:Q], AF.Exp)
                nc.vector.tensor_copy(xall_sb[:, Q * k:Q * (k + 1)], px[:, 0:Q])
                nc.vector.tensor_copy(stat_sb[:, k, 0:Q], etmp_sb[:, k, :])
                nc.vector.tensor_tensor(stat_sb[:, k, Q:2 * Q], px[:, 0:Q],
                                        etmp_sb[:, k, :], ALU.mult)

            # masked reductions: stats[17, S] += stat_chunk.T @ maskT_chunk
            stats_ps = ps.tile([17, SB, 512], f32, tag="s", bufs=1, name="s")
            for k in range(NCHUNK):
                for b in range(SB):
                    nc.tensor.matmul(stats_ps[:, b, :], stat_sb[:, k, :],
                                     mslice(k, b),
                                     start=(k == 0), stop=(k == NCHUNK - 1))

            # psum->sbuf copies + output DMAs all on ACT: program order makes
            # each DMACopy carry only its single HW-queue ring wait
            for b in range(SB):
                nc.scalar.activation(stats_sb[:, 512 * b:512 * (b + 1)],
                                     stats_ps[:, b, :], AF.Copy)
            nc.scalar.dma_start(stats_d[:], stats_sb[:])
            nc.scalar.dma_start(xall_d[:], xall_sb[:])

    nc.finalize()
    return nc


def _get_prog():
    global _PROG
    if _PROG is None:
        _PROG = _build_prog()
    return _PROG


def _prep(feature_vecs, W1, b1, W2, b2, keys, mask):
    m8 = mask.view(np.uint8) if mask.dtype == np.bool_ else mask.astype(np.uint8)
    m8 = m8 * np.uint8(0x38)               # fp8e4m3 bit pattern of 1.0
    mT = np.ascontiguousarray(m8.T)        # [N, S]

    wb = np.zeros((E, E + Q + 2), np.float32)
    wb[:, 0:E] = np.asarray(W2, np.float32).T
    wb[:, E:E + Q] = np.asarray(keys, np.float32).T
    wb[:, E + Q] = np.asarray(b1, np.float32)
    wb[:, E + Q + 1] = np.asarray(b2, np.float32)

    in_maps = []
    for d in range(NC):
        sl = slice(d * NPC, (d + 1) * NPC)
        fw = np.zeros((F, E + NPAD), np.float32)
        fw[:, 0:E] = np.asarray(W1, np.float32)
        fw[:, E:E + NPC] = feature_vecs[sl].T
        mt = np.zeros((NPAD, S), np.uint8)
        mt[:NPC] = mT[sl]
        mt4 = np.ascontiguousarray(
            mt.reshape(NSC, 2, 128, S).transpose(2, 0, 1, 3))
        in_maps.append({
            "fw": fw,
            "wb": wb,
            "maskT": mt4.view(ml_dtypes.float8_e4m3),
        })
    return in_maps


def kernel(feature_vecs, W1, b1, W2, b2, keys, rewards, mask, queue_idx, sel_idx):
    import sys
    if "/opt/trn_rl_repo" not in sys.path:
        sys.path.insert(0, "/opt/trn_rl_repo")
    from concourse.bass_utils import run_bass_kernel_spmd

    nc = _get_prog()
    in_maps = _prep(feature_vecs, W1, b1, W2, b2, keys, mask)
    res = run_bass_kernel_spmd(nc, in_maps, list(range(NC))).results

    qs = np.asarray(queue_idx).astype(np.int64)
    ar = np.arange(S)
    Z = np.zeros(S, np.float64)
    S1 = np.zeros(S, np.float64)
    cnt = np.zeros(S, np.float64)
    for d in range(NC):
        st = res[d]["stats"].astype(np.float64)
        Z += st[qs, ar]
        S1 += st[Q + qs, ar]
        cnt += st[16]

    xall = np.stack([res[d]["xall"] for d in range(NC)]).astype(np.float64)
    sel = np.asarray(sel_idx).astype(np.int64)
    d_arr = sel // NPC
    nloc = sel % NPC
    x_sel = xall[d_arr, nloc % 128, (nloc // 128) * Q + qs]

    logZ = np.log(Z)
    ce = logZ - x_sel
    me = (S1 / Z - logZ) / np.log(cnt)
    loss = (np.asarray(rewards, np.float64) * ce).sum() + ENTROPY_COEF * me.sum()
    return np.array([loss], dtype=np.float32)


# revision 27
# speedup vs baseline: 1.3774x; 1.3774x over previous
import numpy as np
import ml_dtypes

N = 50000
F = 64
E = 128
Q = 8
S = 2048
NC = 8
NPC = N // NC          # 6250 clauses per core
NPAD = 6400            # 25 * 256
NSC = 25               # super-chunks of 256 clauses (DoubleRow)
NCHUNK = 50            # 128-chunks
SB = 4                 # psum banks of 512 steps each
ST = 32                # stationary cols: Ehi(8) Elo(8) Ghi(8) Glo(8)
                       # (DoubleRow Ldweights needs col count % 16 == 0;
                       #  counts come from mask.sum on host instead)
GS = 0.125             # scale on x for G so fp8 never saturates (|G|<448)
ENTROPY_COEF = 0.1

_PROG = None


def _build_prog():
    import sys
    if "/opt/trn_rl_repo" not in sys.path:
        sys.path.insert(0, "/opt/trn_rl_repo")
    from concourse import bass, bacc, tile, mybir

    f32 = mybir.dt.float32
    bf16 = mybir.dt.bfloat16
    f8 = mybir.dt.float8e4
    AF = mybir.ActivationFunctionType
    ALU = mybir.AluOpType
    DR = mybir.MatmulPerfMode.DoubleRow

    # Bacc's finalize() runs generate_event_semaphores: splits multi-wait
    # instructions into InstEventSemaphore chains (HW allows 1 wait/inst).
    nc = bacc.Bacc("TRN2")
    # fw = [W1 | fvT] bf16 on 64 partitions; wb = [K2T | b1] f32 on 128.
    # One DMA per matmul-operand group keeps every compute instruction at
    # <=1 sync wait (HW limit; more crashes neuronxcc codegen).
    fw_d = nc.dram_tensor("fw", [F, E + NPAD], bf16, kind="ExternalInput")
    wb_d = nc.dram_tensor("wb", [E, Q + 1], f32, kind="ExternalInput")
    maskT_d = nc.dram_tensor("maskT", [128, NSC, 2, S], f8, kind="ExternalInput")
    stats_d = nc.dram_tensor("stats", [ST, S], f32, kind="ExternalOutput")
    xall_d = nc.dram_tensor("xall", [E, NCHUNK * Q], f32, kind="ExternalOutput")

    with tile.TileContext(nc) as tc:
        with (
            tc.tile_pool(name="const", bufs=1) as constp,
            tc.tile_pool(name="big", bufs=1) as bigp,
            tc.tile_pool(name="mask", bufs=1) as maskp,
            tc.tile_pool(name="ps", bufs=1, space=bass.MemorySpace.PSUM) as ps,
        ):
            wb_sb = constp.tile([E, Q + 1], f32)
            k2t_sb = constp.tile([E, Q], bf16)
            scr_sb = constp.tile([1, 1], f32)

            fw_sb = bigp.tile([F, E + NPAD], bf16)
            ht_sb = bigp.tile([E, NPAD], bf16)
            xall_sb = bigp.tile([E, NCHUNK * Q], f32)
            e_sb = bigp.tile([E, NSC, 2, Q], f32)
            xs_sb = bigp.tile([E, NSC, 2, Q], f32)
            g_sb = bigp.tile([E, NSC, 2, Q], f32)
            ehi_sb = bigp.tile([E, NSC, 2, Q], f32)
            ghi_sb = bigp.tile([E, NSC, 2, Q], f32)
            stat_sb = bigp.tile([E, NSC, 2, ST], f8)
            stats_sb = bigp.tile([ST, SB * 512], f32)

            w1_sb = fw_sb[:, 0:E]
            fvt_sb = fw_sb[:, E:E + NPAD]
            b1_sb = wb_sb[:, Q:Q + 1]

            nc.scalar.dma_start(wb_sb[:], wb_d[:])
            nc.scalar.dma_start(fw_sb[:], fw_d[:])

            # <=8 total DMA instructions -> every DMA gets a unique DMAHW
            # lane, so no lane-reuse wait; each DMACopy carries at most its
            # single data wait (HW limit). Mask split across both physical
            # HWDGE rings (SP + ACT) for bandwidth.
            MB = [(0, 8, nc.sync), (8, 16, nc.sync), (16, NSC, nc.scalar)]
            mts = []
            for s0, s1, eng in MB:
                mt = maskp.tile([128, s1 - s0, 2, S], f8, tag=f"m{s0}")
                eng.dma_start(mt[:], maskT_d[:, s0:s1, :, :])
                mts.append(mt)

            def mslice(sc, b):
                for (s0, s1, _), mt in zip(MB, mts):
                    if s0 <= sc < s1:
                        return mt[:, sc - s0, :, 512 * b:512 * (b + 1)]

            # ACT absorber for the wb DMA semaphore, then k2t copy on ACT so
            # the px matmuls depend on a single engine (ACT) only.
            nc.scalar.activation(scr_sb[:], wb_sb[0:1, 0:1], AF.Relu)
            nc.scalar.activation(k2t_sb[:], wb_sb[:, 0:Q], AF.Copy)

            # hT = relu(W1.T @ fvT + b1)   [E, NPAD] bf16
            for j in range((NPAD + 511) // 512):
                c0 = 512 * j
                cw = min(512, NPAD - c0)
                ph = ps.tile([E, 512], f32, tag="w", bufs=2, name="w")
                nc.tensor.matmul(ph[:, :cw], w1_sb, fvt_sb[:, c0:c0 + cw],
                                 start=True, stop=True)
                nc.scalar.activation(ht_sb[:, c0:c0 + cw], ph[:, :cw], AF.Relu,
                                     bias=b1_sb, scale=1.0)

            # x'[n,q] = hT_chunk.T @ K2T  (c_q dropped: softmax shift-invariant)
            xps = ps.tile([E, 512], f32, tag="x", bufs=1, name="x")
            for k in range(NCHUNK):
                nc.tensor.matmul(xps[:, Q * k:Q * (k + 1)],
                                 ht_sb[:, 128 * k:128 * (k + 1)], k2t_sb[:],
                                 start=True, stop=True)

            nx = NCHUNK * Q
            nc.scalar.activation(e_sb[:], xps[:, 0:nx], AF.Exp)
            nc.scalar.activation(xs_sb[:], xps[:, 0:nx], AF.Copy, scale=GS)
            nc.scalar.activation(xall_sb[:], xps[:, 0:nx], AF.Copy)
            nc.vector.tensor_tensor(g_sb[:], xs_sb[:], e_sb[:], ALU.mult)

            # hi/lo fp8 split: value = hi + lo with ~2^-8 combined rel err
            nc.vector.tensor_copy(stat_sb[:, :, :, 0:Q], e_sb[:])
            nc.vector.tensor_copy(ehi_sb[:], stat_sb[:, :, :, 0:Q])
            nc.vector.tensor_tensor(stat_sb[:, :, :, Q:2 * Q], e_sb[:],
                                    ehi_sb[:], ALU.subtract)
            nc.vector.tensor_copy(stat_sb[:, :, :, 2 * Q:3 * Q], g_sb[:])
            nc.vector.tensor_copy(ghi_sb[:], stat_sb[:, :, :, 2 * Q:3 * Q])
            nc.vector.tensor_tensor(stat_sb[:, :, :, 3 * Q:4 * Q], g_sb[:],
                                    ghi_sb[:], ALU.subtract)

            stats_ps = ps.tile([ST, SB, 512], f32, tag="s", bufs=1, name="s")

            # stats[33, S] += stat_chunk.T @ maskT_chunk, fp8 DoubleRow (K=256)
            for k in range(NSC):
                for b in range(SB):
                    nc.tensor.matmul(stats_ps[:, b, :], stat_sb[:, k, :, :],
                                     mslice(k, b),
                                     start=(k == 0), stop=(k == NSC - 1),
                                     perf_mode=DR, skip_group_check=True)

            # psum->sbuf copies + output DMAs all on ACT: program order makes
            # each DMACopy carry only its single HW-queue ring wait
            for b in range(SB):
                nc.scalar.activation(stats_sb[:, 512 * b:512 * (b + 1)],
                                     stats_ps[:, b, :], AF.Copy)
            nc.scalar.dma_start(stats_d[:], stats_sb[:])
            nc.scalar.dma_start(xall_d[:], xall_sb[:])

    nc.finalize()
    return nc


def _get_prog():
    global _PROG
    if _PROG is None:
        _PROG = _build_prog()
    return _PROG


def _prep(feature_vecs, W1, b1, W2, b2, keys, mask):
    m8 = mask.view(np.uint8) if mask.dtype == np.bool_ else mask.astype(np.uint8)
    m8 = m8 * np.uint8(0x38)               # fp8e4m3 bit pattern of 1.0
    mT = np.ascontiguousarray(m8.T)        # [N, S]

    wb = np.zeros((E, Q + 1), np.float32)
    wb[:, 0:Q] = (np.asarray(W2, np.float64) @ np.asarray(keys, np.float64).T
                  ).astype(np.float32)     # K2T[e,q]
    wb[:, Q] = np.asarray(b1, np.float32)

    w1b = np.asarray(W1).astype(ml_dtypes.bfloat16)

    in_maps = []
    for d in range(NC):
        sl = slice(d * NPC, (d + 1) * NPC)
        fw = np.zeros((F, E + NPAD), ml_dtypes.bfloat16)
        fw[:, 0:E] = w1b
        fw[:, E:E + NPC] = feature_vecs[sl].T.astype(ml_dtypes.bfloat16)
        mt = np.zeros((NPAD, S), np.uint8)
        mt[:NPC] = mT[sl]
        mt4 = np.ascontiguousarray(
            mt.reshape(NSC, 2, 128, S).transpose(2, 0, 1, 3))
        in_maps.append({
            "fw": fw,
            "wb": wb,
            "maskT": mt4.view(ml_dtypes.float8_e4m3),
        })
    return in_maps


def kernel(feature_vecs, W1, b1, W2, b2, keys, rewards, mask, queue_idx, sel_idx):
    import sys
    if "/opt/trn_rl_repo" not in sys.path:
        sys.path.insert(0, "/opt/trn_rl_repo")
    from concourse.bass_utils import run_bass_kernel_spmd

    nc = _get_prog()
    in_maps = _prep(feature_vecs, W1, b1, W2, b2, keys, mask)
    res = run_bass_kernel_spmd(nc, in_maps, list(range(NC))).results

    qs = np.asarray(queue_idx).astype(np.int64)
    ar = np.arange(S)
    Z = np.zeros(S, np.float64)
    S1 = np.zeros(S, np.float64)
    cnt = np.asarray(mask).sum(axis=1, dtype=np.float64)
    for d in range(NC):
        st = res[d]["stats"].astype(np.float64)
        Z += st[qs, ar] + st[Q + qs, ar]
        S1 += st[2 * Q + qs, ar] + st[3 * Q + qs, ar]
    S1 /= GS

    xall = np.stack([res[d]["xall"] for d in range(NC)]).astype(np.float64)
    sel = np.asarray(sel_idx).astype(np.int64)
    d_arr = sel // NPC
    nloc = sel % NPC
    x_sel = xall[d_arr, nloc % 128, (nloc // 128) * Q + qs]

    logZ = np.log(Z)
    ce = logZ - x_sel
    me = (S1 / Z - logZ) / np.log(cnt)
    loss = (np.asarray(rewards, np.float64) * ce).sum() + ENTROPY_COEF * me.sum()
    return np.array([loss], dtype=np.float32)
